# revision 11
# baseline (speedup 1.0000x reference)
"""BitLinear inference kernel for Trainium2, SPMD over 8 NeuronCores.

Reference computation (per batch b of x[b] @ [T, D], kernel [D, F]):
  x_norm  = x * rsqrt(mean(x^2, -1) + 1e-5)
  x_scale = 127 / clip(max|x_norm|, 1e-5)          (per row)
  x_quant = round(x_norm * x_scale).clip(-128,127) / x_scale
  w_scale = mean|kernel|.clip(1e-5)
  w_quant = sign(kernel - mean(kernel)) * w_scale
  out     = (x_quant @ w_quant) / w_scale / x_scale

Algebra: w_scale cancels exactly, and with
  q    = round(x * 127/max|x|)    (integers in [-127, 127])
  s    = sign(kernel - mean(kernel))  (+-1)
  out  = (q @ s) * (max|x|^2 / (127^2 * (mean(x^2)+1e-5)))   per row.
q and s are exactly representable in bf16/fp8 and the PE accumulates in
fp32, so the matmul is exact integer arithmetic.

v3 structure:
  * s lives entirely in SBUF as two half tiles [P, KB, 2048] fp8 — no
    DRAM bounce (keeps HBM traffic low enough that the chip never hits
    the firmware power throttle that capped the old kernel at 1.95GHz).
  * w is read in [128 x 8KiB-row] wide tiles (~4x the effective DMA
    rate of 2KiB-row reads).  Half 0 signs are produced on the DVE as
    (w > mean) - 0.5 (exact in fp8; compensated by doubling the post
    scale for those chunks), half 1 on the ACT engine as sign(w-mean),
    so neither engine serializes production.
  * The global mean is an AllReduce of per-core row-slice partials; it
    lands ~75us in (runtime barrier dominated).  Phase 1 then runs two
    k-lockstep sweeps across 8 PSUM banks (2 token blocks x 4 chunks)
    so consumption tracks sign production tile-by-tile, followed by
    resident sweeps for token blocks 2-3.
  * Steady state streams one 128-token block at a time: quant chain on
    the DVE, XBAR transpose on the SP ring, 8 chunk accumulation
    groups, DVE PSUM drains (ACT never sits between sign production
    and drains).
Sharding: data-parallel — one batch of x per core, kernel replicated.
"""

import re
from contextlib import ExitStack

import numpy as np

import concourse.bass as bass
import concourse.mybir as mybir
import concourse.tile as tile
from concourse.tile import ScopedClock, VectorClock


# ---------------------------------------------------------------------------
# The walrus build in this container only accepts a single sync-wait per
# Drain instruction; TileContext's tail drain carries one wait per live
# semaphore.  Split it into one drain per semaphore.
# ---------------------------------------------------------------------------
def _drain_and_barrier_split(self, tick_clock, wait_clock):
    m = re.search(r"VectorClock\(\[([^\]]*)\]\)", repr(tick_clock.global_clock))
    vals = [int(v) for v in m.group(1).split(",")]
    procs = [(i, v) for i, v in enumerate(vals) if v > 0]
    for i, v in procs or [(0, 0)]:
        sub = VectorClock()
        sub.require_at_least(i, v)
        drain_inst = self.nc.sync.drain()
        wait_clock.add_sem_waits(drain_inst.ins, ScopedClock({None: sub}))

    self.nc.all_engine_barrier()
    assert self.sems is not None
    popped = self.nc._tile_sem_poison_stack.pop()
    assert popped is self._sem_poison
    self.nc.clear_and_free_semaphores(list(self.sems.allocated().values()))
    self.nc.all_engine_barrier()


def install_drain_patch():
    tile.TileContext._drain_and_barrier = _drain_and_barrier_split


def split_multi_waits(nc: bass.Bass, max_waits: int = 1):
    """The walrus in this container accepts at most one sync-wait per
    instruction.  Hoist extra waits onto NoOps injected just before the
    instruction on the same engine (engines execute their stream in order,
    so waiting on A then B sequentially == waiting on both)."""
    n_split = 0
    for fn in nc.m.functions:
        for bb in fn.blocks:
            insts = bb.instructions
            if not any(
                ins.sync_info is not None and len(ins.sync_info.on_wait or []) > max_waits
                for ins in insts
            ):
                continue
            out = []
            for ins in insts:
                si = ins.sync_info
                if si is not None and len(si.on_wait or []) > max_waits:
                    waits = list(si.on_wait)
                    for j, w in enumerate(waits[:-max_waits]):
                        nop = mybir.InstNoOp(name=f"{ins.name}-wsplit{j}", ins=[], outs=[])
                        nop.engine = ins.engine
                        nop.sync_info = mybir.SyncInfo(on_wait=[w], on_update=[])
                        nc.register_instruction(nop, overwrite=True)
                        out.append(nop)
                    ins.sync_info = mybir.SyncInfo(
                        on_wait=waits[-max_waits:], on_update=list(si.on_update or [])
                    )
                    n_split += 1
                out.append(ins)
            bb.instructions = out
    return n_split


MAGIC = float(1.5 * 2.0**23)  # keeps v+MAGIC in [2^23, 2^24) for |v| <= 2^22 -> RNE to integer
F32 = mybir.dt.float32
BF16 = mybir.dt.bfloat16
FP8 = mybir.dt.float8e4
P = 128


def build_bitlinear(nc: bass.Bass, T=4096, D=4096, F=4096, FC=512, world=8):
    """Emit the per-core program: x [T, D] f32, w [D, F] f32 -> out [T, F] f32.

    wslice [D/world, F] is this core's row-slice of w; partial sums are
    AllReduced so each core only reads 1/world of w for the global mean."""
    AF = mybir.ActivationFunctionType
    OP = mybir.AluOpType
    KB = D // P          # contraction slices
    NB = T // P          # 128-token blocks
    NFC = F // FC        # output-feature chunks
    WC = F // 2          # wide w tile / s half width
    CPH = NFC // 2       # chunks per half
    SUB = 512            # bn_stats subgroup width
    HD = D // 2          # x half-tile width
    NSUB = D // SUB
    WR = D // world      # wslice rows

    x_in = nc.dram_tensor("x", [T, D], F32, kind="ExternalInput")
    w_in = nc.dram_tensor("w", [D, F], F32, kind="ExternalInput")
    ws_in = nc.dram_tensor("wslice", [WR, F], F32, kind="ExternalInput")
    out = nc.dram_tensor("out", [T, F], F32, kind="ExternalOutput")

    with tile.TileContext(nc) as tc, ExitStack() as ctx:
        xp = ctx.enter_context(tc.tile_pool(name="xp", bufs=2))
        qbp = ctx.enter_context(tc.tile_pool(name="qbp", bufs=2))
        qtp = ctx.enter_context(tc.tile_pool(name="qtp", bufs=4))
        wcp = ctx.enter_context(tc.tile_pool(name="wcp", bufs=2))
        sres = ctx.enter_context(tc.tile_pool(name="sres", bufs=1))
        stg = ctx.enter_context(tc.tile_pool(name="stg", bufs=2))
        st = ctx.enter_context(tc.tile_pool(name="st", bufs=4))
        postp = ctx.enter_context(tc.tile_pool(name="postp", bufs=4))
        singles = ctx.enter_context(tc.tile_pool(name="singles", bufs=1))
        psmm = ctx.enter_context(tc.tile_pool(name="psmm", bufs=8, space="PSUM"))
        dram = ctx.enter_context(tc.tile_pool(name="dram", bufs=1, space="DRAM"))

        # ---- W pass 1: global mean via per-core partial sums + AllReduce ----
        NWS = (WR // P) * (F // HD)
        colsum = singles.tile([P, NWS], F32)
        for rb in range(WR // P):
            for hh in range(F // HD):
                wt = xp.tile([P, HD], F32, tag="xt", name="wt")
                nc.scalar.dma_start(
                    out=wt, in_=ws_in[rb * P:(rb + 1) * P, hh * HD:(hh + 1) * HD]
                )
                i = (F // HD) * rb + hh
                nc.vector.reduce_sum(
                    out=colsum[:, i:i + 1], in_=wt, axis=mybir.AxisListType.X
                )
        rowsum_loc = singles.tile([P, 1], F32)
        nc.vector.reduce_sum(out=rowsum_loc, in_=colsum, axis=mybir.AxisListType.X)
        if world > 1:
            cc_in = dram.tile([P, 1], F32, name="cc_in")
            cc_out = dram.tile([P, 1], F32, name="cc_out", addr_space="Shared")
            nc.gpsimd.dma_start(out=cc_in[:, :], in_=rowsum_loc)
            nc.gpsimd.collective_compute(
                "AllReduce",
                mybir.AluOpType.add,
                replica_groups=[list(range(world))],
                ins=[cc_in[:, :]],
                outs=[cc_out[:, :]],
            )
            rowsum = st.tile([P, 1], F32)
            nc.gpsimd.dma_start(out=rowsum, in_=cc_out[:, :])
        else:
            rowsum = rowsum_loc
        ones_mat = singles.tile([P, P], F32)
        nc.vector.memset(ones_mat, 1.0)
        # Single matmul: ps_bc[m] = sum_k rowsum[k] (all-ones stationary)
        ps_bc = psmm.tile([P, FC], F32, tag="ps", name="ps_bc")
        nc.tensor.matmul(ps_bc[:, 0:1], lhsT=ones_mat, rhs=rowsum, start=True, stop=True)
        neg_wmean = singles.tile([P, 1], F32)
        nc.scalar.activation(neg_wmean, ps_bc[:, 0:1], AF.Copy, bias=0.0, scale=-1.0 / (D * F))
        wmean_pos = singles.tile([P, 1], F32)
        nc.scalar.activation(wmean_pos, ps_bc[:, 0:1], AF.Copy, bias=0.0, scale=1.0 / (D * F))

        # ---- resident sign matrix: two half tiles [P, KB, 2048] fp8 ----
        s_half = [
            sres.tile([P, KB, WC], FP8, tag=f"s{j}", name=f"s{j}") for j in range(2)
        ]

        def s_slice(fc, k):
            j, c = divmod(fc, CPH)
            return s_half[j][:, k, c * FC:(c + 1) * FC]

        # ---- x load / quant chain / transpose ----
        qts = {}
        posts = {}
        post2s = {}
        xhs = {}

        def emit_xload(tb):
            xh = []
            for h in range(2):
                xt = xp.tile([P, HD], F32, tag="xt")
                nc.sync.dma_start(
                    out=xt, in_=x_in[tb * P:(tb + 1) * P, h * HD:(h + 1) * HD]
                )
                xh.append(xt)
            xhs[tb] = xh

        def emit_quant(tb):
            xh = xhs.pop(tb)
            am2 = st.tile([P, 2], F32)
            for h in range(2):
                nc.vector.tensor_reduce(
                    out=am2[:, h:h + 1], in_=xh[h], axis=mybir.AxisListType.X,
                    op=OP.max, apply_absolute_value=True,
                )
            am = st.tile([P, 1], F32)
            nc.vector.tensor_reduce(
                out=am, in_=am2, axis=mybir.AxisListType.X,
                op=OP.max, apply_absolute_value=False,
            )
            w1 = st.tile([P, 1], F32)
            nc.vector.tensor_scalar(
                out=w1, in0=am, scalar1=1e-30, scalar2=1.0 / 127.0,
                op0=OP.max, op1=OP.mult,
            )
            cc = st.tile([P, 1], F32)
            nc.vector.reciprocal(cc, w1)

            # mean(x^2) via bn_stats — must read xh before the in-place quant
            stats6 = st.tile([P, NSUB, 6], F32)
            for i in range(NSUB):
                h, off = divmod(i * SUB, HD)
                nc.vector.bn_stats(out=stats6[:, i, :], in_=xh[h][:, off:off + SUB])
            mv = st.tile([P, 2], F32)
            nc.vector.bn_aggr(out=mv, in_=stats6)

            # q = round(x * cc) via the magic-number trick (RNE), bf16 out,
            # one XBAR transpose per half: qT[p, h*16+k, t] = q[t, h*HD+k*P+p]
            qT = qtp.tile([P, KB, P], BF16, tag="qT")
            for h in range(2):
                nc.vector.tensor_scalar(
                    out=xh[h], in0=xh[h], scalar1=cc, scalar2=MAGIC,
                    op0=OP.mult, op1=OP.add,
                )
                qb = qbp.tile([P, HD], BF16, tag="qb")
                nc.vector.tensor_scalar_add(qb, xh[h], -MAGIC)
                nc.sync.dma_start_transpose(
                    out=qT[:, h * (KB // 2):(h + 1) * (KB // 2), :], in_=qb
                )
            qts[tb] = qT

            # output scale: post = max|x|^2 / (127^2 * (mean(x^2)+1e-5));
            # post2 = 2*post for the chunks whose s is stored as +-0.5.
            msq = st.tile([P, 1], F32)
            nc.vector.tensor_mul(msq, mv[:, 0:1], mv[:, 0:1])
            v0 = st.tile([P, 1], F32)
            nc.vector.tensor_add(v0, msq, mv[:, 1:2])
            v1 = st.tile([P, 1], F32)
            nc.vector.tensor_scalar_add(v1, v0, 1e-5)
            r2 = st.tile([P, 1], F32)
            nc.vector.reciprocal(r2, v1)
            am2sq = st.tile([P, 1], F32)
            nc.vector.tensor_mul(am2sq, am, am)
            a2 = st.tile([P, 1], F32)
            nc.vector.tensor_mul(a2, am2sq, r2)
            post = postp.tile([P, 1], F32, tag="post")
            nc.vector.tensor_scalar(
                out=post, in0=a2, scalar1=1e-10, scalar2=1.0 / (127.0 * 127.0),
                op0=OP.max, op1=OP.mult,
            )
            post2 = postp.tile([P, 1], F32, tag="post2")
            nc.vector.tensor_add(post2, post, post)
            posts[tb] = post
            post2s[tb] = post2

        def drain(ps, tb, fc):
            so = stg.tile([P, FC], F32)
            scale = post2s[tb] if fc < CPH else posts[tb]
            nc.vector.tensor_scalar_mul(so, ps, scale)
            nc.sync.dma_start(
                out=out[tb * P:(tb + 1) * P, fc * FC:(fc + 1) * FC], in_=so
            )

        def emit_group(tb, fc):
            ps = psmm.tile([P, FC], F32, tag="ps", name="ps")
            qT = qts[tb]
            for k in range(KB):
                nc.tensor.matmul(
                    ps, lhsT=qT[:, k, :], rhs=s_slice(fc, k),
                    start=(k == 0), stop=(k == KB - 1),
                )
            drain(ps, tb, fc)

        def emit_lockstep(tbs, fcs):
            # 8 PSUM banks accumulate in k-lockstep so consumption tracks
            # sign production tile by tile during the startup window.
            banks = {}
            for tb in tbs:
                for fc in fcs:
                    banks[(tb, fc)] = psmm.tile([P, FC], F32, tag="ps", name="ps")
            for k in range(KB):
                for tb in tbs:
                    for fc in fcs:
                        nc.tensor.matmul(
                            banks[(tb, fc)], lhsT=qts[tb][:, k, :],
                            rhs=s_slice(fc, k),
                            start=(k == 0), stop=(k == KB - 1),
                        )
            for tb in tbs:
                for fc in fcs:
                    drain(banks[(tb, fc)], tb, fc)

        # ---- emission schedule ----
        for tb in range(4):
            emit_xload(tb)

        for tb in range(4):
            emit_quant(tb)
        emit_xload(4)
        emit_xload(5)

        # sign producer: half 0 on the DVE as (w > mean) - 0.5, half 1 on
        # the ACT engine as sign(w - mean); w streams in wide tiles on the
        # ACT ring.
        for j in range(2):
            for kb in range(KB):
                wt2 = wcp.tile([P, WC], F32, tag="wc", name="wt2")
                # Two DMA rings feed the (~200GB/s-per-ring packet-rate
                # limited) w stream: evens on the ACT HWDGE ring, odds on
                # the GPSIMD SWDGE queue, which sits behind the AllReduce
                # and so starts exactly when the mean-gated signs can run.
                ring = nc.scalar if kb % 2 == 0 else nc.gpsimd
                ring.dma_start(
                    out=wt2, in_=w_in[kb * P:(kb + 1) * P, j * WC:(j + 1) * WC]
                )
                if j == 0:
                    nc.vector.tensor_scalar(
                        out=s_half[0][:, kb, :], in0=wt2, scalar1=wmean_pos,
                        scalar2=0.5, op0=OP.is_gt, op1=OP.subtract,
                    )
                else:
                    nc.scalar.activation(
                        out=s_half[1][:, kb, :], in_=wt2, func=AF.Sign,
                        bias=neg_wmean, scale=1.0,
                    )

        # phase 1: chase half 0 with two lockstep token blocks, then the
        # resident sweeps; same for half 1.
        emit_lockstep([0, 1], [0, 1, 2, 3])
        for fc in range(CPH):
            for tb in (2, 3):
                emit_group(tb, fc)
        emit_lockstep([0, 1], [4, 5, 6, 7])
        emit_quant(4)
        emit_quant(5)
        for fc in range(CPH, NFC):
            for tb in (2, 3):
                emit_group(tb, fc)

        # steady state: one token block at a time, next block's quant ahead
        # of this block's drains in the DVE stream.
        for tb in range(4, NB):
            nxt = tb + 2
            if nxt < NB:
                emit_xload(nxt)
            if tb + 1 < NB and (tb + 1) not in qts:
                emit_quant(tb + 1)
            for fc in range(NFC):
                emit_group(tb, fc)
            del qts[tb]
    return nc


_N_CORES = 8
_BATCH = 8
_T = 4096
_D = 4096
_F = 4096


def _ensure_axon_hooks_module():
    """bass_utils imports antenv.axon_hooks when BASS_TRACE is set; the
    module is absent in this image.  Provide a stub so tracing degrades
    gracefully instead of crashing (a real hook may already be installed)."""
    import sys
    import types

    try:
        import antenv.axon_hooks  # noqa: F401
    except ImportError:
        mod = types.ModuleType("antenv.axon_hooks")
        mod._hook = None
        mod.set_axon_ntff_profile_hook = lambda h: setattr(mod, "_hook", h)
        mod.get_axon_ntff_profile_hook = lambda: mod._hook
        sys.modules["antenv.axon_hooks"] = mod


def kernel(x: np.ndarray, kernel: np.ndarray) -> np.ndarray:
    from concourse.bass_utils import run_bass_kernel_spmd

    _ensure_axon_hooks_module()
    install_drain_patch()
    nc = bass.Bass()
    build_bitlinear(nc, T=_T, D=_D, F=_F, FC=512, world=_N_CORES)
    split_multi_waits(nc)

    x = np.ascontiguousarray(np.asarray(x, dtype=np.float32))
    w = np.ascontiguousarray(np.asarray(kernel, dtype=np.float32))
    assert x.shape == (_BATCH, _T, _D) and w.shape == (_D, _F)

    wr = _D // _N_CORES
    in_maps = [
        {
            "x": x[b],
            "w": w,
            "wslice": np.ascontiguousarray(w[b * wr:(b + 1) * wr, :]),
        }
        for b in range(_N_CORES)
    ]
    res = run_bass_kernel_spmd(nc, in_maps, list(range(_N_CORES)))
    global _last_results
    _last_results = res
    return np.stack([res.results[i]["out"] for i in range(_N_CORES)], axis=0)


_last_results = None


# revision 13
# speedup vs baseline: 1.0000x; 1.0000x over previous
"""BitLinear inference kernel for Trainium2, SPMD over 8 NeuronCores.

Reference computation (per batch b of x[b] @ [T, D], kernel [D, F]):
  x_norm  = x * rsqrt(mean(x^2, -1) + 1e-5)
  x_scale = 127 / clip(max|x_norm|, 1e-5)          (per row)
  x_quant = round(x_norm * x_scale).clip(-128,127) / x_scale
  w_scale = mean|kernel|.clip(1e-5)
  w_quant = sign(kernel - mean(kernel)) * w_scale
  out     = (x_quant @ w_quant) / w_scale / x_scale

Algebra: w_scale cancels exactly, and with
  q    = round(x * 127/max|x|)    (integers in [-127, 127])
  s    = sign(kernel - mean(kernel))  (+-1)
  out  = (q @ s) * (max|x|^2 / (127^2 * (mean(x^2)+1e-5)))   per row.
q and s are exactly representable in bf16/fp8 and the PE accumulates in
fp32, so the matmul is exact integer arithmetic.

v3 structure:
  * s lives entirely in SBUF as two half tiles [P, KB, 2048] fp8 — no
    DRAM bounce (keeps HBM traffic low enough that the chip never hits
    the firmware power throttle that capped the old kernel at 1.95GHz).
  * w is read in [128 x 8KiB-row] wide tiles (~4x the effective DMA
    rate of 2KiB-row reads).  Half 0 signs are produced on the DVE as
    (w > mean) - 0.5 (exact in fp8; compensated by doubling the post
    scale for those chunks), half 1 on the ACT engine as sign(w-mean),
    so neither engine serializes production.
  * The global mean is an AllReduce of per-core row-slice partials; it
    lands ~75us in (runtime barrier dominated).  Phase 1 then runs two
    k-lockstep sweeps across 8 PSUM banks (2 token blocks x 4 chunks)
    so consumption tracks sign production tile-by-tile, followed by
    resident sweeps for token blocks 2-3.
  * Steady state streams one 128-token block at a time: quant chain on
    the DVE, XBAR transpose on the SP ring, 8 chunk accumulation
    groups, DVE PSUM drains (ACT never sits between sign production
    and drains).
Sharding: data-parallel — one batch of x per core, kernel replicated.
"""

import re
from contextlib import ExitStack

import numpy as np

import concourse.bass as bass
import concourse.mybir as mybir
import concourse.tile as tile
from concourse.tile import ScopedClock, VectorClock


# ---------------------------------------------------------------------------
# The walrus build in this container only accepts a single sync-wait per
# Drain instruction; TileContext's tail drain carries one wait per live
# semaphore.  Split it into one drain per semaphore.
# ---------------------------------------------------------------------------
def _drain_and_barrier_split(self, tick_clock, wait_clock):
    m = re.search(r"VectorClock\(\[([^\]]*)\]\)", repr(tick_clock.global_clock))
    vals = [int(v) for v in m.group(1).split(",")]
    procs = [(i, v) for i, v in enumerate(vals) if v > 0]
    for i, v in procs or [(0, 0)]:
        sub = VectorClock()
        sub.require_at_least(i, v)
        drain_inst = self.nc.sync.drain()
        wait_clock.add_sem_waits(drain_inst.ins, ScopedClock({None: sub}))

    self.nc.all_engine_barrier()
    assert self.sems is not None
    popped = self.nc._tile_sem_poison_stack.pop()
    assert popped is self._sem_poison
    self.nc.clear_and_free_semaphores(list(self.sems.allocated().values()))
    self.nc.all_engine_barrier()


def install_drain_patch():
    tile.TileContext._drain_and_barrier = _drain_and_barrier_split


def split_multi_waits(nc: bass.Bass, max_waits: int = 1):
    """The walrus in this container accepts at most one sync-wait per
    instruction.  Hoist extra waits onto NoOps injected just before the
    instruction on the same engine (engines execute their stream in order,
    so waiting on A then B sequentially == waiting on both)."""
    n_split = 0
    for fn in nc.m.functions:
        for bb in fn.blocks:
            insts = bb.instructions
            if not any(
                ins.sync_info is not None and len(ins.sync_info.on_wait or []) > max_waits
                for ins in insts
            ):
                continue
            out = []
            for ins in insts:
                si = ins.sync_info
                if si is not None and len(si.on_wait or []) > max_waits:
                    waits = list(si.on_wait)
                    for j, w in enumerate(waits[:-max_waits]):
                        nop = mybir.InstNoOp(name=f"{ins.name}-wsplit{j}", ins=[], outs=[])
                        nop.engine = ins.engine
                        nop.sync_info = mybir.SyncInfo(on_wait=[w], on_update=[])
                        nc.register_instruction(nop, overwrite=True)
                        out.append(nop)
                    ins.sync_info = mybir.SyncInfo(
                        on_wait=waits[-max_waits:], on_update=list(si.on_update or [])
                    )
                    n_split += 1
                out.append(ins)
            bb.instructions = out
    return n_split


MAGIC = float(1.5 * 2.0**23)  # keeps v+MAGIC in [2^23, 2^24) for |v| <= 2^22 -> RNE to integer
F32 = mybir.dt.float32
BF16 = mybir.dt.bfloat16
FP8 = mybir.dt.float8e4
P = 128


def build_bitlinear(nc: bass.Bass, T=4096, D=4096, F=4096, FC=512, world=8):
    """Emit the per-core program: x [T, D] f32, w [D, F] f32 -> out [T, F] f32.

    wslice [D/world, F] is this core's row-slice of w; partial sums are
    AllReduced so each core only reads 1/world of w for the global mean."""
    AF = mybir.ActivationFunctionType
    OP = mybir.AluOpType
    KB = D // P          # contraction slices
    NB = T // P          # 128-token blocks
    NFC = F // FC        # output-feature chunks
    WC = F // 2          # wide w tile / s half width
    CPH = NFC // 2       # chunks per half
    SUB = 512            # bn_stats subgroup width
    HD = D // 2          # x half-tile width
    NSUB = D // SUB
    WR = D // world      # wslice rows

    x_in = nc.dram_tensor("x", [T, D], F32, kind="ExternalInput")
    w_in = nc.dram_tensor("w", [D, F], F32, kind="ExternalInput")
    ws_in = nc.dram_tensor("wslice", [WR, F], F32, kind="ExternalInput")
    out = nc.dram_tensor("out", [T, F], F32, kind="ExternalOutput")

    with tile.TileContext(nc) as tc, ExitStack() as ctx:
        xp = ctx.enter_context(tc.tile_pool(name="xp", bufs=2))
        qbp = ctx.enter_context(tc.tile_pool(name="qbp", bufs=1))
        qtp = ctx.enter_context(tc.tile_pool(name="qtp", bufs=4))
        wcp = ctx.enter_context(tc.tile_pool(name="wcp", bufs=5))
        sres = ctx.enter_context(tc.tile_pool(name="sres", bufs=1))
        stg = ctx.enter_context(tc.tile_pool(name="stg", bufs=2))
        st = ctx.enter_context(tc.tile_pool(name="st", bufs=4))
        postp = ctx.enter_context(tc.tile_pool(name="postp", bufs=4))
        singles = ctx.enter_context(tc.tile_pool(name="singles", bufs=1))
        psmm = ctx.enter_context(tc.tile_pool(name="psmm", bufs=8, space="PSUM"))
        dram = ctx.enter_context(tc.tile_pool(name="dram", bufs=1, space="DRAM"))

        # ---- W pass 1: global mean via per-core partial sums + AllReduce ----
        NWS = (WR // P) * (F // HD)
        colsum = singles.tile([P, NWS], F32)
        for rb in range(WR // P):
            for hh in range(F // HD):
                wt = xp.tile([P, HD], F32, tag="xt", name="wt")
                nc.scalar.dma_start(
                    out=wt, in_=ws_in[rb * P:(rb + 1) * P, hh * HD:(hh + 1) * HD]
                )
                i = (F // HD) * rb + hh
                nc.vector.reduce_sum(
                    out=colsum[:, i:i + 1], in_=wt, axis=mybir.AxisListType.X
                )
        rowsum_loc = singles.tile([P, 1], F32)
        nc.vector.reduce_sum(out=rowsum_loc, in_=colsum, axis=mybir.AxisListType.X)
        if world > 1:
            cc_in = dram.tile([P, 1], F32, name="cc_in")
            cc_out = dram.tile([P, 1], F32, name="cc_out", addr_space="Shared")
            nc.gpsimd.dma_start(out=cc_in[:, :], in_=rowsum_loc)
            nc.gpsimd.collective_compute(
                "AllReduce",
                mybir.AluOpType.add,
                replica_groups=[list(range(world))],
                ins=[cc_in[:, :]],
                outs=[cc_out[:, :]],
            )
            rowsum = st.tile([P, 1], F32)
            nc.gpsimd.dma_start(out=rowsum, in_=cc_out[:, :])
        else:
            rowsum = rowsum_loc
        ones_mat = singles.tile([P, P], F32)
        nc.vector.memset(ones_mat, 1.0)
        # Single matmul: ps_bc[m] = sum_k rowsum[k] (all-ones stationary)
        ps_bc = psmm.tile([P, FC], F32, tag="ps", name="ps_bc")
        nc.tensor.matmul(ps_bc[:, 0:1], lhsT=ones_mat, rhs=rowsum, start=True, stop=True)
        neg_wmean = singles.tile([P, 1], F32)
        nc.scalar.activation(neg_wmean, ps_bc[:, 0:1], AF.Copy, bias=0.0, scale=-1.0 / (D * F))
        wmean_pos = singles.tile([P, 1], F32)
        nc.scalar.activation(wmean_pos, ps_bc[:, 0:1], AF.Copy, bias=0.0, scale=1.0 / (D * F))

        # ---- resident sign matrix: two half tiles [P, KB, 2048] fp8 ----
        s_half = [
            sres.tile([P, KB, WC], FP8, tag=f"s{j}", name=f"s{j}") for j in range(2)
        ]

        def s_slice(fc, k):
            j, c = divmod(fc, CPH)
            return s_half[j][:, k, c * FC:(c + 1) * FC]

        # ---- x load / quant chain / transpose ----
        qts = {}
        posts = {}
        post2s = {}
        xhs = {}

        def emit_xload(tb):
            xh = []
            for h in range(2):
                xt = xp.tile([P, HD], F32, tag="xt")
                nc.sync.dma_start(
                    out=xt, in_=x_in[tb * P:(tb + 1) * P, h * HD:(h + 1) * HD]
                )
                xh.append(xt)
            xhs[tb] = xh

        def emit_quant(tb):
            xh = xhs.pop(tb)
            am2 = st.tile([P, 2], F32)
            for h in range(2):
                nc.vector.tensor_reduce(
                    out=am2[:, h:h + 1], in_=xh[h], axis=mybir.AxisListType.X,
                    op=OP.max, apply_absolute_value=True,
                )
            am = st.tile([P, 1], F32)
            nc.vector.tensor_reduce(
                out=am, in_=am2, axis=mybir.AxisListType.X,
                op=OP.max, apply_absolute_value=False,
            )
            w1 = st.tile([P, 1], F32)
            nc.vector.tensor_scalar(
                out=w1, in0=am, scalar1=1e-30, scalar2=1.0 / 127.0,
                op0=OP.max, op1=OP.mult,
            )
            cc = st.tile([P, 1], F32)
            nc.vector.reciprocal(cc, w1)

            # mean(x^2) via bn_stats — must read xh before the in-place quant
            stats6 = st.tile([P, NSUB, 6], F32)
            for i in range(NSUB):
                h, off = divmod(i * SUB, HD)
                nc.vector.bn_stats(out=stats6[:, i, :], in_=xh[h][:, off:off + SUB])
            mv = st.tile([P, 2], F32)
            nc.vector.bn_aggr(out=mv, in_=stats6)

            # q = round(x * cc) via the magic-number trick (RNE), bf16 out,
            # one XBAR transpose per half: qT[p, h*16+k, t] = q[t, h*HD+k*P+p]
            qT = qtp.tile([P, KB, P], BF16, tag="qT")
            for h in range(2):
                nc.vector.tensor_scalar(
                    out=xh[h], in0=xh[h], scalar1=cc, scalar2=MAGIC,
                    op0=OP.mult, op1=OP.add,
                )
                qb = qbp.tile([P, HD], BF16, tag="qb")
                nc.vector.tensor_scalar_add(qb, xh[h], -MAGIC)
                nc.sync.dma_start_transpose(
                    out=qT[:, h * (KB // 2):(h + 1) * (KB // 2), :], in_=qb
                )
            qts[tb] = qT

            # output scale: post = max|x|^2 / (127^2 * (mean(x^2)+1e-5));
            # post2 = 2*post for the chunks whose s is stored as +-0.5.
            msq = st.tile([P, 1], F32)
            nc.vector.tensor_mul(msq, mv[:, 0:1], mv[:, 0:1])
            v0 = st.tile([P, 1], F32)
            nc.vector.tensor_add(v0, msq, mv[:, 1:2])
            v1 = st.tile([P, 1], F32)
            nc.vector.tensor_scalar_add(v1, v0, 1e-5)
            r2 = st.tile([P, 1], F32)
            nc.vector.reciprocal(r2, v1)
            am2sq = st.tile([P, 1], F32)
            nc.vector.tensor_mul(am2sq, am, am)
            a2 = st.tile([P, 1], F32)
            nc.vector.tensor_mul(a2, am2sq, r2)
            post = postp.tile([P, 1], F32, tag="post")
            nc.vector.tensor_scalar(
                out=post, in0=a2, scalar1=1e-10, scalar2=1.0 / (127.0 * 127.0),
                op0=OP.max, op1=OP.mult,
            )
            post2 = postp.tile([P, 1], F32, tag="post2")
            nc.vector.tensor_add(post2, post, post)
            posts[tb] = post
            post2s[tb] = post2

        def drain(ps, tb, fc):
            so = stg.tile([P, FC], F32)
            scale = post2s[tb] if fc < CPH else posts[tb]
            nc.vector.tensor_scalar_mul(so, ps, scale)
            nc.sync.dma_start(
                out=out[tb * P:(tb + 1) * P, fc * FC:(fc + 1) * FC], in_=so
            )

        def emit_group(tb, fc):
            ps = psmm.tile([P, FC], F32, tag="ps", name="ps")
            qT = qts[tb]
            for k in range(KB):
                nc.tensor.matmul(
                    ps, lhsT=qT[:, k, :], rhs=s_slice(fc, k),
                    start=(k == 0), stop=(k == KB - 1),
                )
            drain(ps, tb, fc)

        def emit_lockstep(tbs, fcs):
            # 8 PSUM banks accumulate in k-lockstep so consumption tracks
            # sign production tile by tile during the startup window.
            banks = {}
            for tb in tbs:
                for fc in fcs:
                    banks[(tb, fc)] = psmm.tile([P, FC], F32, tag="ps", name="ps")
            for k in range(KB):
                for tb in tbs:
                    for fc in fcs:
                        nc.tensor.matmul(
                            banks[(tb, fc)], lhsT=qts[tb][:, k, :],
                            rhs=s_slice(fc, k),
                            start=(k == 0), stop=(k == KB - 1),
                        )
            for tb in tbs:
                for fc in fcs:
                    drain(banks[(tb, fc)], tb, fc)

        # ---- emission schedule ----
        for tb in range(4):
            emit_xload(tb)

        for tb in range(4):
            emit_quant(tb)
        emit_xload(4)
        emit_xload(5)

        # sign producer: half 0 on the DVE as (w > mean) - 0.5, half 1 on
        # the ACT engine as sign(w - mean); w streams in wide tiles on the
        # ACT ring.
        # Two DMA rings feed the w stream in half-width tiles: the per-slot
        # release->restart latency (~5us) hides behind the 5-deep ring, and
        # alternating ACT-HWDGE / GPSIMD-SWDGE doubles the packet rate.
        # The GPSIMD queue sits behind the AllReduce, so its share starts
        # exactly when the mean-gated signs can run.
        WC2 = WC // 2
        for j in range(2):
            for kb in range(KB):
                for h in range(2):
                    wt2 = wcp.tile([P, WC2], F32, tag="wc", name="wt2")
                    ring = nc.scalar if (2 * kb + h) % 2 == 0 else nc.gpsimd
                    c0 = j * WC + h * WC2
                    ring.dma_start(
                        out=wt2, in_=w_in[kb * P:(kb + 1) * P, c0:c0 + WC2]
                    )
                    if j == 0:
                        nc.vector.tensor_scalar(
                            out=s_half[0][:, kb, h * WC2:(h + 1) * WC2],
                            in0=wt2, scalar1=wmean_pos,
                            scalar2=0.5, op0=OP.is_gt, op1=OP.subtract,
                        )
                    else:
                        nc.scalar.activation(
                            out=s_half[1][:, kb, h * WC2:(h + 1) * WC2],
                            in_=wt2, func=AF.Sign,
                            bias=neg_wmean, scale=1.0,
                        )

        # phase 1: chase half 0 with two lockstep token blocks, then the
        # resident sweeps; same for half 1.
        emit_lockstep([0, 1], [0, 1, 2, 3])
        for fc in range(CPH):
            for tb in (2, 3):
                emit_group(tb, fc)
        emit_lockstep([0, 1], [4, 5, 6, 7])
        emit_quant(4)
        emit_quant(5)
        for fc in range(CPH, NFC):
            for tb in (2, 3):
                emit_group(tb, fc)

        # steady state: one token block at a time, next block's quant ahead
        # of this block's drains in the DVE stream.
        for tb in range(4, NB):
            nxt = tb + 2
            if nxt < NB:
                emit_xload(nxt)
            if tb + 1 < NB and (tb + 1) not in qts:
                emit_quant(tb + 1)
            for fc in range(NFC):
                emit_group(tb, fc)
            del qts[tb]
    return nc


_N_CORES = 8
_BATCH = 8
_T = 4096
_D = 4096
_F = 4096


def _ensure_axon_hooks_module():
    """bass_utils imports antenv.axon_hooks when BASS_TRACE is set; the
    module is absent in this image.  Provide a stub so tracing degrades
    gracefully instead of crashing (a real hook may already be installed)."""
    import sys
    import types

    try:
        import antenv.axon_hooks  # noqa: F401
    except ImportError:
        mod = types.ModuleType("antenv.axon_hooks")
        mod._hook = None
        mod.set_axon_ntff_profile_hook = lambda h: setattr(mod, "_hook", h)
        mod.get_axon_ntff_profile_hook = lambda: mod._hook
        sys.modules["antenv.axon_hooks"] = mod


def kernel(x: np.ndarray, kernel: np.ndarray) -> np.ndarray:
    from concourse.bass_utils import run_bass_kernel_spmd

    _ensure_axon_hooks_module()
    install_drain_patch()
    nc = bass.Bass()
    build_bitlinear(nc, T=_T, D=_D, F=_F, FC=512, world=_N_CORES)
    split_multi_waits(nc)

    x = np.ascontiguousarray(np.asarray(x, dtype=np.float32))
    w = np.ascontiguousarray(np.asarray(kernel, dtype=np.float32))
    assert x.shape == (_BATCH, _T, _D) and w.shape == (_D, _F)

    wr = _D // _N_CORES
    in_maps = [
        {
            "x": x[b],
            "w": w,
            "wslice": np.ascontiguousarray(w[b * wr:(b + 1) * wr, :]),
        }
        for b in range(_N_CORES)
    ]
    res = run_bass_kernel_spmd(nc, in_maps, list(range(_N_CORES)))
    global _last_results
    _last_results = res
    return np.stack([res.results[i]["out"] for i in range(_N_CORES)], axis=0)


_last_results = None


# revision 15
# speedup vs baseline: 1.0235x; 1.0234x over previous
"""BitLinear inference kernel for Trainium2, SPMD over 8 NeuronCores.

Reference computation (per batch b of x[b] @ [T, D], kernel [D, F]):
  x_norm  = x * rsqrt(mean(x^2, -1) + 1e-5)
  x_scale = 127 / clip(max|x_norm|, 1e-5)          (per row)
  x_quant = round(x_norm * x_scale).clip(-128,127) / x_scale
  w_scale = mean|kernel|.clip(1e-5)
  w_quant = sign(kernel - mean(kernel)) * w_scale
  out     = (x_quant @ w_quant) / w_scale / x_scale

Algebra: w_scale cancels exactly, and with
  q    = round(x * 127/max|x|)    (integers in [-127, 127])
  s    = sign(kernel - mean(kernel))  (+-1)
  out  = (q @ s) * (max|x|^2 / (127^2 * (mean(x^2)+1e-5)))   per row.
q and s are exactly representable in bf16/fp8 and the PE accumulates in
fp32, so the matmul is exact integer arithmetic.

v3 structure:
  * s lives entirely in SBUF as two half tiles [P, KB, 2048] fp8 — no
    DRAM bounce (keeps HBM traffic low enough that the chip never hits
    the firmware power throttle that capped the old kernel at 1.95GHz).
  * w is read in [128 x 8KiB-row] wide tiles (~4x the effective DMA
    rate of 2KiB-row reads).  Half 0 signs are produced on the DVE as
    (w > mean) - 0.5 (exact in fp8; compensated by doubling the post
    scale for those chunks), half 1 on the ACT engine as sign(w-mean),
    so neither engine serializes production.
  * The global mean is an AllReduce of per-core row-slice partials; it
    lands ~75us in (runtime barrier dominated).  Phase 1 then runs two
    k-lockstep sweeps across 8 PSUM banks (2 token blocks x 4 chunks)
    so consumption tracks sign production tile-by-tile, followed by
    resident sweeps for token blocks 2-3.
  * Steady state streams one 128-token block at a time: quant chain on
    the DVE, XBAR transpose on the SP ring, 8 chunk accumulation
    groups, DVE PSUM drains (ACT never sits between sign production
    and drains).
Sharding: data-parallel — one batch of x per core, kernel replicated.
"""

import re
from contextlib import ExitStack

import numpy as np

import concourse.bass as bass
import concourse.mybir as mybir
import concourse.tile as tile
from concourse.tile import ScopedClock, VectorClock


# ---------------------------------------------------------------------------
# The walrus build in this container only accepts a single sync-wait per
# Drain instruction; TileContext's tail drain carries one wait per live
# semaphore.  Split it into one drain per semaphore.
# ---------------------------------------------------------------------------
def _drain_and_barrier_split(self, tick_clock, wait_clock):
    m = re.search(r"VectorClock\(\[([^\]]*)\]\)", repr(tick_clock.global_clock))
    vals = [int(v) for v in m.group(1).split(",")]
    procs = [(i, v) for i, v in enumerate(vals) if v > 0]
    for i, v in procs or [(0, 0)]:
        sub = VectorClock()
        sub.require_at_least(i, v)
        drain_inst = self.nc.sync.drain()
        wait_clock.add_sem_waits(drain_inst.ins, ScopedClock({None: sub}))

    self.nc.all_engine_barrier()
    assert self.sems is not None
    popped = self.nc._tile_sem_poison_stack.pop()
    assert popped is self._sem_poison
    self.nc.clear_and_free_semaphores(list(self.sems.allocated().values()))
    self.nc.all_engine_barrier()


def install_drain_patch():
    tile.TileContext._drain_and_barrier = _drain_and_barrier_split


def split_multi_waits(nc: bass.Bass, max_waits: int = 1):
    """The walrus in this container accepts at most one sync-wait per
    instruction.  Hoist extra waits onto NoOps injected just before the
    instruction on the same engine (engines execute their stream in order,
    so waiting on A then B sequentially == waiting on both)."""
    n_split = 0
    for fn in nc.m.functions:
        for bb in fn.blocks:
            insts = bb.instructions
            if not any(
                ins.sync_info is not None and len(ins.sync_info.on_wait or []) > max_waits
                for ins in insts
            ):
                continue
            out = []
            for ins in insts:
                si = ins.sync_info
                if si is not None and len(si.on_wait or []) > max_waits:
                    waits = list(si.on_wait)
                    for j, w in enumerate(waits[:-max_waits]):
                        nop = mybir.InstNoOp(name=f"{ins.name}-wsplit{j}", ins=[], outs=[])
                        nop.engine = ins.engine
                        nop.sync_info = mybir.SyncInfo(on_wait=[w], on_update=[])
                        nc.register_instruction(nop, overwrite=True)
                        out.append(nop)
                    ins.sync_info = mybir.SyncInfo(
                        on_wait=waits[-max_waits:], on_update=list(si.on_update or [])
                    )
                    n_split += 1
                out.append(ins)
            bb.instructions = out
    return n_split


MAGIC = float(1.5 * 2.0**23)  # keeps v+MAGIC in [2^23, 2^24) for |v| <= 2^22 -> RNE to integer
F32 = mybir.dt.float32
BF16 = mybir.dt.bfloat16
FP8 = mybir.dt.float8e4
P = 128


def build_bitlinear(nc: bass.Bass, T=4096, D=4096, F=4096, FC=512, world=8):
    """Emit the per-core program: x [T, D] f32, w [D, F] f32 -> out [T, F] f32.

    wslice [D/world, F] is this core's row-slice of w; partial sums are
    AllReduced so each core only reads 1/world of w for the global mean."""
    AF = mybir.ActivationFunctionType
    OP = mybir.AluOpType
    KB = D // P          # contraction slices
    NB = T // P          # 128-token blocks
    NFC = F // FC        # output-feature chunks
    WC = F // 2          # wide w tile / s half width
    CPH = NFC // 2       # chunks per half
    SUB = 512            # bn_stats subgroup width
    HD = D // 2          # x half-tile width
    NSUB = D // SUB
    WR = D // world      # wslice rows

    x_in = nc.dram_tensor("x", [T, D], F32, kind="ExternalInput")
    w_in = nc.dram_tensor("w", [D, F], F32, kind="ExternalInput")
    ws_in = nc.dram_tensor("wslice", [WR, F], F32, kind="ExternalInput")
    out = nc.dram_tensor("out", [T, F], F32, kind="ExternalOutput")

    with tile.TileContext(nc) as tc, ExitStack() as ctx:
        xp = ctx.enter_context(tc.tile_pool(name="xp", bufs=2))
        qbp = ctx.enter_context(tc.tile_pool(name="qbp", bufs=2))
        qtp = ctx.enter_context(tc.tile_pool(name="qtp", bufs=4))
        wcp = ctx.enter_context(tc.tile_pool(name="wcp", bufs=2))
        sres = ctx.enter_context(tc.tile_pool(name="sres", bufs=1))
        stg = ctx.enter_context(tc.tile_pool(name="stg", bufs=2))
        st = ctx.enter_context(tc.tile_pool(name="st", bufs=4))
        postp = ctx.enter_context(tc.tile_pool(name="postp", bufs=4))
        singles = ctx.enter_context(tc.tile_pool(name="singles", bufs=1))
        psmm = ctx.enter_context(tc.tile_pool(name="psmm", bufs=8, space="PSUM"))
        dram = ctx.enter_context(tc.tile_pool(name="dram", bufs=1, space="DRAM"))

        # ---- W pass 1: global mean via per-core partial sums + AllReduce ----
        NWS = (WR // P) * (F // HD)
        colsum = singles.tile([P, NWS], F32)
        for rb in range(WR // P):
            for hh in range(F // HD):
                wt = xp.tile([P, HD], F32, tag="xt", name="wt")
                nc.scalar.dma_start(
                    out=wt, in_=ws_in[rb * P:(rb + 1) * P, hh * HD:(hh + 1) * HD]
                )
                i = (F // HD) * rb + hh
                nc.vector.reduce_sum(
                    out=colsum[:, i:i + 1], in_=wt, axis=mybir.AxisListType.X
                )
        rowsum_loc = singles.tile([P, 1], F32)
        nc.vector.reduce_sum(out=rowsum_loc, in_=colsum, axis=mybir.AxisListType.X)
        if world > 1:
            cc_in = dram.tile([P, 1], F32, name="cc_in")
            cc_out = dram.tile([P, 1], F32, name="cc_out", addr_space="Shared")
            nc.gpsimd.dma_start(out=cc_in[:, :], in_=rowsum_loc)
            nc.gpsimd.collective_compute(
                "AllReduce",
                mybir.AluOpType.add,
                replica_groups=[list(range(world))],
                ins=[cc_in[:, :]],
                outs=[cc_out[:, :]],
            )
            rowsum = st.tile([P, 1], F32)
            nc.gpsimd.dma_start(out=rowsum, in_=cc_out[:, :])
        else:
            rowsum = rowsum_loc
        ones_mat = singles.tile([P, P], F32)
        nc.vector.memset(ones_mat, 1.0)
        # Single matmul: ps_bc[m] = sum_k rowsum[k] (all-ones stationary)
        ps_bc = psmm.tile([P, FC], F32, tag="ps", name="ps_bc")
        nc.tensor.matmul(ps_bc[:, 0:1], lhsT=ones_mat, rhs=rowsum, start=True, stop=True)
        neg_wmean = singles.tile([P, 1], F32)
        nc.scalar.activation(neg_wmean, ps_bc[:, 0:1], AF.Copy, bias=0.0, scale=-1.0 / (D * F))
        wmean_pos = singles.tile([P, 1], F32)
        nc.scalar.activation(wmean_pos, ps_bc[:, 0:1], AF.Copy, bias=0.0, scale=1.0 / (D * F))

        # ---- resident sign matrix: two half tiles [P, KB, 2048] fp8 ----
        s_half = [
            sres.tile([P, KB, WC], FP8, tag=f"s{j}", name=f"s{j}") for j in range(2)
        ]

        def s_slice(fc, k):
            j, c = divmod(fc, CPH)
            return s_half[j][:, k, c * FC:(c + 1) * FC]

        # ---- x load / quant chain / transpose ----
        qts = {}
        posts = {}
        post2s = {}
        xhs = {}

        def emit_xload(tb):
            xh = []
            for h in range(2):
                xt = xp.tile([P, HD], F32, tag="xt")
                nc.sync.dma_start(
                    out=xt, in_=x_in[tb * P:(tb + 1) * P, h * HD:(h + 1) * HD]
                )
                xh.append(xt)
            xhs[tb] = xh

        def emit_quant(tb):
            xh = xhs.pop(tb)
            am2 = st.tile([P, 2], F32)
            for h in range(2):
                nc.vector.tensor_reduce(
                    out=am2[:, h:h + 1], in_=xh[h], axis=mybir.AxisListType.X,
                    op=OP.max, apply_absolute_value=True,
                )
            am = st.tile([P, 1], F32)
            nc.vector.tensor_reduce(
                out=am, in_=am2, axis=mybir.AxisListType.X,
                op=OP.max, apply_absolute_value=False,
            )
            w1 = st.tile([P, 1], F32)
            nc.vector.tensor_scalar(
                out=w1, in0=am, scalar1=1e-30, scalar2=1.0 / 127.0,
                op0=OP.max, op1=OP.mult,
            )
            cc = st.tile([P, 1], F32)
            nc.vector.reciprocal(cc, w1)

            # mean(x^2) via bn_stats — must read xh before the in-place quant
            stats6 = st.tile([P, NSUB, 6], F32)
            for i in range(NSUB):
                h, off = divmod(i * SUB, HD)
                nc.vector.bn_stats(out=stats6[:, i, :], in_=xh[h][:, off:off + SUB])
            mv = st.tile([P, 2], F32)
            nc.vector.bn_aggr(out=mv, in_=stats6)

            # q = round(x * cc) via the magic-number trick (RNE), bf16 out,
            # one XBAR transpose per half: qT[p, h*16+k, t] = q[t, h*HD+k*P+p]
            qT = qtp.tile([P, KB, P], BF16, tag="qT")
            for h in range(2):
                nc.vector.tensor_scalar(
                    out=xh[h], in0=xh[h], scalar1=cc, scalar2=MAGIC,
                    op0=OP.mult, op1=OP.add,
                )
                qb = qbp.tile([P, HD], BF16, tag="qb")
                nc.vector.tensor_scalar_add(qb, xh[h], -MAGIC)
                nc.sync.dma_start_transpose(
                    out=qT[:, h * (KB // 2):(h + 1) * (KB // 2), :], in_=qb
                )
            qts[tb] = qT

            # output scale: post = max|x|^2 / (127^2 * (mean(x^2)+1e-5));
            # post2 = 2*post for the chunks whose s is stored as +-0.5.
            msq = st.tile([P, 1], F32)
            nc.vector.tensor_mul(msq, mv[:, 0:1], mv[:, 0:1])
            v0 = st.tile([P, 1], F32)
            nc.vector.tensor_add(v0, msq, mv[:, 1:2])
            v1 = st.tile([P, 1], F32)
            nc.vector.tensor_scalar_add(v1, v0, 1e-5)
            r2 = st.tile([P, 1], F32)
            nc.vector.reciprocal(r2, v1)
            am2sq = st.tile([P, 1], F32)
            nc.vector.tensor_mul(am2sq, am, am)
            a2 = st.tile([P, 1], F32)
            nc.vector.tensor_mul(a2, am2sq, r2)
            post = postp.tile([P, 1], F32, tag="post")
            nc.vector.tensor_scalar(
                out=post, in0=a2, scalar1=1e-10, scalar2=1.0 / (127.0 * 127.0),
                op0=OP.max, op1=OP.mult,
            )
            post2 = postp.tile([P, 1], F32, tag="post2")
            nc.vector.tensor_add(post2, post, post)
            posts[tb] = post
            post2s[tb] = post2

        def drain(ps, tb, fc):
            so = stg.tile([P, FC], F32)
            scale = post2s[tb] if fc < CPH else posts[tb]
            nc.vector.tensor_scalar_mul(so, ps, scale)
            nc.sync.dma_start(
                out=out[tb * P:(tb + 1) * P, fc * FC:(fc + 1) * FC], in_=so
            )

        def emit_group(tb, fc):
            ps = psmm.tile([P, FC], F32, tag="ps", name="ps")
            qT = qts[tb]
            for k in range(KB):
                nc.tensor.matmul(
                    ps, lhsT=qT[:, k, :], rhs=s_slice(fc, k),
                    start=(k == 0), stop=(k == KB - 1),
                )
            drain(ps, tb, fc)

        def emit_lockstep(tbs, fcs):
            # 8 PSUM banks accumulate in k-lockstep so consumption tracks
            # sign production tile by tile during the startup window.
            banks = {}
            for tb in tbs:
                for fc in fcs:
                    banks[(tb, fc)] = psmm.tile([P, FC], F32, tag="ps", name="ps")
            for k in range(KB):
                for tb in tbs:
                    for fc in fcs:
                        nc.tensor.matmul(
                            banks[(tb, fc)], lhsT=qts[tb][:, k, :],
                            rhs=s_slice(fc, k),
                            start=(k == 0), stop=(k == KB - 1),
                        )
            for tb in tbs:
                for fc in fcs:
                    drain(banks[(tb, fc)], tb, fc)

        # ---- emission schedule ----
        for tb in range(4):
            emit_xload(tb)

        for tb in range(4):
            emit_quant(tb)
        emit_xload(4)
        emit_xload(5)

        # sign producer: half 0 on the DVE as (w > mean) - 0.5, half 1 on
        # the ACT engine as sign(w - mean); w streams in wide tiles on the
        # ACT ring.
        for j in range(2):
            for kb in range(KB):
                wt2 = wcp.tile([P, WC], F32, tag="wc", name="wt2")
                nc.scalar.dma_start(
                    out=wt2, in_=w_in[kb * P:(kb + 1) * P, j * WC:(j + 1) * WC]
                )
                if j == 0:
                    nc.vector.tensor_scalar(
                        out=s_half[0][:, kb, :], in0=wt2, scalar1=wmean_pos,
                        scalar2=0.5, op0=OP.is_gt, op1=OP.subtract,
                    )
                else:
                    nc.scalar.activation(
                        out=s_half[1][:, kb, :], in_=wt2, func=AF.Sign,
                        bias=neg_wmean, scale=1.0,
                    )

        # phase 1: chase half 0 with two lockstep token blocks, then the
        # resident sweeps; same for half 1.
        emit_lockstep([0, 1], [0, 1, 2, 3])
        for fc in range(CPH):
            for tb in (2, 3):
                emit_group(tb, fc)
        emit_lockstep([0, 1], [4, 5, 6, 7])
        emit_quant(4)
        emit_quant(5)
        for fc in range(CPH, NFC):
            for tb in (2, 3):
                emit_group(tb, fc)

        # steady state: one token block at a time, next block's quant ahead
        # of this block's drains in the DVE stream.
        for tb in range(4, NB):
            nxt = tb + 2
            if nxt < NB:
                emit_xload(nxt)
            if tb + 1 < NB and (tb + 1) not in qts:
                emit_quant(tb + 1)
            for fc in range(NFC):
                emit_group(tb, fc)
            del qts[tb]
    return nc


_N_CORES = 8
_BATCH = 8
_T = 4096
_D = 4096
_F = 4096


def _ensure_axon_hooks_module():
    """bass_utils imports antenv.axon_hooks when BASS_TRACE is set; the
    module is absent in this image.  Provide a stub so tracing degrades
    gracefully instead of crashing (a real hook may already be installed)."""
    import sys
    import types

    try:
        import antenv.axon_hooks  # noqa: F401
    except ImportError:
        mod = types.ModuleType("antenv.axon_hooks")
        mod._hook = None
        mod.set_axon_ntff_profile_hook = lambda h: setattr(mod, "_hook", h)
        mod.get_axon_ntff_profile_hook = lambda: mod._hook
        sys.modules["antenv.axon_hooks"] = mod


def kernel(x: np.ndarray, kernel: np.ndarray) -> np.ndarray:
    from concourse.bass_utils import run_bass_kernel_spmd

    _ensure_axon_hooks_module()
    install_drain_patch()
    nc = bass.Bass()
    build_bitlinear(nc, T=_T, D=_D, F=_F, FC=512, world=_N_CORES)
    split_multi_waits(nc)

    x = np.ascontiguousarray(np.asarray(x, dtype=np.float32))
    w = np.ascontiguousarray(np.asarray(kernel, dtype=np.float32))
    assert x.shape == (_BATCH, _T, _D) and w.shape == (_D, _F)

    wr = _D // _N_CORES
    in_maps = [
        {
            "x": x[b],
            "w": w,
            "wslice": np.ascontiguousarray(w[b * wr:(b + 1) * wr, :]),
        }
        for b in range(_N_CORES)
    ]
    res = run_bass_kernel_spmd(nc, in_maps, list(range(_N_CORES)))
    global _last_results
    _last_results = res
    return np.stack([res.results[i]["out"] for i in range(_N_CORES)], axis=0)


_last_results = None
